# revision 19
# baseline (speedup 1.0000x reference)
"""Distributed 2-layer GCN for Trainium2 (8 NeuronCores).

Math (matches the reference):
    x   = embed[tok] @ Wn.T + bn
    deg = in-degree over (edges + self loops); dinv = 1/sqrt(deg)
    per layer l (W, b):   h = x @ W.T
                          z[d] = sum_{e: dst=d} dinv[src] dinv[d] h[src]
                          out  = z + b ; relu between layers
Decomposition:  g = dinv * (x @ W.T) row table; z[d] = dinv[d] * sum g[src].
Layer-1 folds the prep matmul (A1 = W1 @ Wn).  Between layers
p1 = dinv^2 * relu(z1) (dinv commutes with relu), g2 = p1 @ W2.T.

Sharding: nodes by contiguous blocks of 12500 per core (dst ownership).
Each core computes g1 for its shard; AllGather (NPIECE pieces, Shared
pair-HBM outputs) -> full table; edges bucketed by (dst window, table
region<=32767 rows for int16 idx); dma_gather pulls message rows; one-hot
S built on DVE scatter-adds 128 msgs per TensorE matmul into per-window
PSUM.  Posts run on the Activation engine; writes are batched per group
via 4-D DRAM tiles (DRAM row = call/group-local permutation).
"""
import sys
import numpy as np

sys.path.insert(0, "/opt/trn_rl_repo")

import ml_dtypes
import concourse.bass as bass
import concourse.bacc as bacc
import concourse.mybir as mybir
import concourse.tile as tile
from concourse.bass_utils import run_bass_kernel_spmd

BF = ml_dtypes.bfloat16

# ---------------- configuration ----------------

class Cfg:
    def __init__(self, N, E, V, DIN, D, NC=8, PREP_CALL=512, CALLBLK=8, SGW=6,
                 NPIECE=2):
        self.N, self.E, self.V, self.DIN, self.D = N, E, V, DIN, D
        self.NC = NC
        self.NPC = N // NC
        self.SGW = SGW                         # windows per group (1 PSUM bank each)
        # pad dst windows so groups are whole and split evenly into pieces
        nw = -(-self.NPC // 128)
        nw = -(-nw // SGW) * SGW
        while (nw // SGW) % NPIECE:
            nw += SGW
        self.NW = nw
        self.DST_ROWS = self.NW * 128
        self.NGRP = self.NW // SGW
        self.HALF = min(((V // 2 + 127) // 128) * 128, 32767 - 128)
        assert self.HALF <= 32767 and V - self.HALF <= 32767
        self.PREP_CALL = PREP_CALL             # idx per embed-gather call
        self.TPC = PREP_CALL // 128
        self.CALLBLK = CALLBLK                 # max 128-msg blocks per gather call
        self.NREG = 4                          # int16 table regions
        self.NPIECE = NPIECE                   # AllGather pieces
        self.AG_SPACE = "Local"                # "Shared" pair-HBM crashes this runtime
        assert self.NREG % NPIECE == 0
        self.RPP = self.NREG // NPIECE         # regions per piece
        assert self.NGRP % NPIECE == 0
        self.NQ = 4                            # SWDGE queues


FULL = Cfg(N=100000, E=1600000, V=50000, DIN=256, D=128)


def _wrap_idx16(idx_linear):
    """dma_gather index layout: slot i -> [i%16, i//16]; [128, n/16] int16."""
    n = idx_linear.shape[0]
    assert n % 16 == 0
    arr = idx_linear.astype(np.int16).reshape(n // 16, 16).T
    return np.ascontiguousarray(np.tile(arr, (8, 1)))


# ---------------- host preprocessing ----------------

class Prep:
    pass


def _region_of(cfg, d_row, core, rows_per_core):
    """(region, y) for table DRAM row d_row of `core`; rows_per_core local rows."""
    pr_pp = rows_per_core // cfg.NPIECE
    piece_rows = cfg.NC * pr_pp
    reg_sz = piece_rows // cfg.RPP
    assert reg_sz <= 32767, reg_sz
    P = d_row // pr_pp
    dp = d_row % pr_pp
    flat = core * pr_pp + dp
    return P * cfg.RPP + flat // reg_sz, flat % reg_sz


def preprocess(cfg, node_tokens, edge_index):
    c = cfg
    tok = np.asarray(node_tokens).astype(np.int64).ravel()
    ei = np.asarray(edge_index).astype(np.int64)
    src, dst = ei[0], ei[1]

    deg = np.bincount(dst, minlength=c.N).astype(np.float64) + 1.0
    dinv = (1.0 / np.sqrt(deg)).astype(np.float32)

    # --- per-core node orderings
    pos_p = np.empty(c.N, np.int64)   # prep slot (gather order, vocab-half grouped)
    pos_d = np.empty(c.N, np.int64)   # dst rank (degree-sorted windows)
    realA = np.empty(c.NC, np.int64)
    per_core = []
    for k in range(c.NC):
        g0 = k * c.NPC
        nodes = np.arange(g0, g0 + c.NPC)
        t = tok[nodes]
        isB = t >= c.HALF
        nA = int((~isB).sum())
        realA[k] = nA
        ordp = np.argsort(isB, kind="stable")
        per_core.append((nodes, t, isB, ordp, nA))

    rnd = c.PREP_CALL
    NA = int(-(-(realA.max()) // rnd) * rnd)
    NBmax = int((c.NPC - realA).max())
    NB_ROWS = int(-(-NBmax // rnd) * rnd)
    PREP_ROWS = NA + NB_ROWS
    # total calls must split evenly into NPIECE pieces
    while (PREP_ROWS // rnd) % c.NPIECE:
        PREP_ROWS += rnd
        NB_ROWS += rnd
    PREP_TILES = PREP_ROWS // 128
    N_CALLS = PREP_ROWS // rnd

    p = Prep()
    p.cfg = c
    p.NA, p.PREP_ROWS, p.PREP_TILES, p.N_CALLS = NA, PREP_ROWS, PREP_TILES, N_CALLS
    p.REG1 = (c.NC * PREP_ROWS // c.NPIECE) // c.RPP
    p.REG2 = (c.NC * c.DST_ROWS // c.NPIECE) // c.RPP

    prep_idx = np.zeros((c.NC, PREP_ROWS), np.int64)
    dinv_p = np.zeros((c.NC, PREP_ROWS), np.float32)
    dinv_d = np.zeros((c.NC, c.DST_ROWS), np.float32)
    order_d = np.zeros((c.NC, c.NPC), np.int64)
    for k in range(c.NC):
        nodes, t, isB, ordp, nA = per_core[k]
        slots = np.empty(c.NPC, np.int64)
        slots[ordp[:nA]] = np.arange(nA)
        slots[ordp[nA:]] = NA + np.arange(c.NPC - nA)
        pos_p[nodes] = slots
        prep_idx[k][slots] = np.where(isB, t - c.HALF, t)
        dinv_p[k][slots] = dinv[nodes]
        od = np.argsort(-deg[nodes], kind="stable")
        rank = np.empty(c.NPC, np.int64)
        rank[od] = np.arange(c.NPC)
        pos_d[nodes] = rank
        order_d[k] = od
        dinv_d[k][rank] = dinv[nodes]

    core_of = np.arange(c.N) // c.NPC

    # prep DRAM row of slot s: call cc, tile j, partition pp -> cc*PC + pp*TPC + j
    s = pos_p
    d1 = (s // c.PREP_CALL) * c.PREP_CALL + (s % 128) * c.TPC + (s % c.PREP_CALL) // 128
    # dst DRAM row of rank: group g, window wg, partition pp -> g*SGW*128 + pp*SGW + wg
    w_of = pos_d // 128
    col_of = pos_d % 128
    d2 = (w_of // c.SGW) * (c.SGW * 128) + col_of * c.SGW + (w_of % c.SGW)

    def reg_y(d_rows, rows_per_core):
        pr_pp = rows_per_core // c.NPIECE
        piece_rows = c.NC * pr_pp
        reg_sz = piece_rows // c.RPP
        assert reg_sz <= 32767, reg_sz
        P = d_rows // pr_pp
        dp = d_rows % pr_pp
        flat = core_of * pr_pp + dp
        return P * c.RPP + flat // reg_sz, flat % reg_sz

    reg1, y1 = reg_y(d1, PREP_ROWS)
    reg2, y2 = reg_y(d2, c.DST_ROWS)

    # --- edges incl self loops
    es = np.concatenate([src, np.arange(c.N)])
    ed = np.concatenate([dst, np.arange(c.N)])
    ecore = ed // c.NPC
    e_dl = pos_d[ed]                 # local dst rank (within owner core)
    e_w = e_dl // 128
    e_col = (e_dl % 128).astype(np.float32)

    def layer_streams(e_r_all, e_i_all):
        """Per-core merged idx+col stream + global (shared) block schedule."""
        e_r = e_r_all[es]
        e_i = e_i_all[es]
        cnt = np.zeros((c.NC, c.NW, c.NREG), np.int64)
        per_core_order = []
        for k in range(c.NC):
            m = ecore == k
            wk, rk, ik, colk = e_w[m], e_r[m], e_i[m], e_col[m]
            o = np.lexsort((colk, rk, wk))
            wk, rk, ik, colk = wk[o], rk[o], ik[o], colk[o]
            cnt[k] = np.bincount(wk * c.NREG + rk, minlength=c.NW * c.NREG).reshape(c.NW, c.NREG)
            per_core_order.append((wk, rk, ik, colk))
        cmax = cnt.max(axis=0)                # [NW, NREG] shared (max-over-core)
        nblk = np.maximum(-(-cmax // 128), 1)
        groups = [list(range(g * c.SGW, (g + 1) * c.SGW)) for g in range(c.NGRP)]
        blk_w = []
        base = np.zeros((c.NW, c.NREG), np.int64)
        calls = []          # (grp_idx, region, b0, nbc, nreg_live)
        bid = 0
        for gi, grp in enumerate(groups):
            # piece-major region order, rotated within piece for balance
            r_order = [P * c.RPP + (gi + j) % c.RPP
                       for P in range(c.NPIECE) for j in range(c.RPP)]
            for r in r_order:
                run0 = bid
                for w in grp:
                    base[w, r] = bid
                    nb = int(nblk[w, r])
                    blk_w.extend([w] * nb)
                    bid += nb
                b0 = run0
                while b0 < bid:
                    nbc = min(c.CALLBLK, bid - b0)
                    tail_w = blk_w[b0 + nbc - 1]
                    ntail = 0
                    if b0 + nbc == bid or blk_w[b0 + nbc] != tail_w:
                        ntail = int(nblk[tail_w, r] * 128 - cmax[tail_w, r])
                    ntail = (ntail // 16) * 16
                    ntail = min(ntail, nbc * 128 - 16)
                    calls.append((gi, r, b0, nbc, nbc * 128 - ntail))
                    b0 += nbc
        TOTBLK = bid
        blk_w = np.array(blk_w)
        last_blk, first_blk = {}, {}
        for b, w in enumerate(blk_w):
            last_blk[int(w)] = b
            if int(w) not in first_blk:
                first_blk[int(w)] = b
        # merged per-core streams: per call [idx nbc*8 | col-bf16 nbc] int16 cols
        offs = []
        off = 0
        for (_gi, _r, _b0, _nbc, _nl) in calls:
            offs.append(off)
            off += _nbc * 10
        STREAM_COLS = off
        streams = []
        for k in range(c.NC):
            wk, rk, ik, colk = per_core_order[k]
            gid = wk * c.NREG + rk
            starts = np.zeros(c.NW * c.NREG, np.int64)
            cc2 = np.bincount(gid, minlength=c.NW * c.NREG)
            starts[1:] = np.cumsum(cc2)[:-1]
            posin = np.arange(len(gid)) - starts[gid]
            slot = base.reshape(-1)[gid] * 128 + posin
            idx_flat = np.zeros(TOTBLK * 128, np.int64)
            col_flat = np.full(TOTBLK * 128, -1.0, np.float32)
            idx_flat[slot] = ik
            col_flat[slot] = colk
            stream = np.zeros((128, STREAM_COLS), np.int16)
            for ci, (_gi, _r, _b0, _nbc, _nl) in enumerate(calls):
                if _nl < _nbc * 128:
                    idx_flat[_b0 * 128 + _nl:(_b0 + _nbc) * 128] = -1
                o = offs[ci]
                stream[:, o:o + _nbc * 8] = _wrap_idx16(
                    idx_flat[_b0 * 128:(_b0 + _nbc) * 128])
                colb = col_flat[_b0 * 128:(_b0 + _nbc) * 128].reshape(_nbc, 128).T
                stream[:, o + _nbc * 8:o + _nbc * 10] = (
                    np.ascontiguousarray(colb).view(np.int16))
            streams.append(np.ascontiguousarray(stream))
        sched = dict(groups=groups, calls=calls, offs=offs, blk_w=blk_w,
                     last_blk=last_blk, first_blk=first_blk, TOTBLK=TOTBLK,
                     STREAM_COLS=STREAM_COLS)
        return sched, streams

    p.s1, p.st1 = layer_streams(reg1, y1)
    p.s2, p.st2 = layer_streams(reg2, y2)
    p.prep_idx = [_wrap_idx16(prep_idx[k]) for k in range(c.NC)]
    p.dinv_p = [np.ascontiguousarray(dinv_p[k].reshape(PREP_TILES, 128).T) for k in range(c.NC)]
    p.dinv_d = [np.ascontiguousarray(dinv_d[k].reshape(c.NW, 128).T) for k in range(c.NC)]
    p.order_d = order_d
    return p


# ---------------- device kernel ----------------

def build_nc(p):
    c = p.cfg
    f32, bf16, i16 = mybir.dt.float32, mybir.dt.bfloat16, mybir.dt.int16
    D, DIN, TPC, SGW = c.D, c.DIN, c.TPC, c.SGW
    nc = bacc.Bacc("TRN2", target_bir_lowering=False, debug=False,
                   num_devices=c.NC, num_swdge_queues=c.NQ)

    embed = nc.dram_tensor("embed", [c.V, DIN], f32, kind="ExternalInput").ap()
    prep_idx_d = nc.dram_tensor("prep_idx", [128, p.PREP_ROWS // 16], i16, kind="ExternalInput").ap()
    st1_d = nc.dram_tensor("st1", [128, p.s1["STREAM_COLS"]], i16, kind="ExternalInput").ap()
    st2_d = nc.dram_tensor("st2", [128, p.s2["STREAM_COLS"]], i16, kind="ExternalInput").ap()
    dinvp_d = nc.dram_tensor("dinvp", [128, p.PREP_TILES], f32, kind="ExternalInput").ap()
    dinvd_d = nc.dram_tensor("dinvd", [128, c.NW], f32, kind="ExternalInput").ap()
    dinvd2_d = nc.dram_tensor("dinvd2", [128, c.NW], f32, kind="ExternalInput").ap()
    a1t_d = nc.dram_tensor("a1t", [DIN, D], bf16, kind="ExternalInput").ap()
    w2t_d = nc.dram_tensor("w2t", [D, D], bf16, kind="ExternalInput").ap()
    iota_d = nc.dram_tensor("iota", [128, D], bf16, kind="ExternalInput").ap()
    ident_d = nc.dram_tensor("ident", [128, 128], bf16, kind="ExternalInput").ap()
    outp = nc.dram_tensor("out", [c.NGRP, 128, SGW, D], f32, kind="ExternalOutput").ap()

    KIN = DIN // 128
    CALLS_PP = p.N_CALLS // c.NPIECE
    GRP_PP = c.NGRP // c.NPIECE
    relu = mybir.ActivationFunctionType.Relu
    act_copy = mybir.ActivationFunctionType.Copy

    with tile.TileContext(nc) as tc:
        with (
            tc.tile_pool(name="dram", bufs=1, space="DRAM") as dpool,
            tc.tile_pool(name="const", bufs=1) as cpool,
            tc.tile_pool(name="io", bufs=6) as iopool,
            tc.tile_pool(name="msg", bufs=3) as msgpool,
            tc.tile_pool(name="emb", bufs=3) as embpool,
            tc.tile_pool(name="s", bufs=8) as spool,
            tc.tile_pool(name="post", bufs=8) as postpool,
            tc.tile_pool(name="grp", bufs=2) as grppool,
            tc.tile_pool(name="zp", bufs=c.SGW, space="PSUM") as zpool,
            tc.tile_pool(name="aux", bufs=2, space="PSUM") as auxpool,
        ):
            g1b = dpool.tile([p.N_CALLS, 128, TPC, D], bf16)
            g2b = dpool.tile([c.NGRP, 128, SGW, D], bf16)
            g1f = [dpool.tile([c.NC * p.PREP_ROWS // c.NPIECE, D], bf16,
                              addr_space=c.AG_SPACE, name=f"g1f{P}")
                   for P in range(c.NPIECE)]
            g2f = [dpool.tile([c.NC * c.DST_ROWS // c.NPIECE, D], bf16,
                              addr_space=c.AG_SPACE, name=f"g2f{P}")
                   for P in range(c.NPIECE)]

            iota_t = cpool.tile([128, D], bf16)
            nc.sync.dma_start(iota_t[:], iota_d[:])
            ident_t = cpool.tile([128, 128], bf16)
            nc.sync.dma_start(ident_t[:], ident_d[:])
            a1t_t = cpool.tile([128, KIN, D], bf16)
            for kk in range(KIN):
                nc.sync.dma_start(a1t_t[:, kk, :], a1t_d[kk * 128:(kk + 1) * 128, :])
            w2t_t = cpool.tile([128, D], bf16)
            nc.sync.dma_start(w2t_t[:], w2t_d[:])
            dinvp_t = cpool.tile([128, p.PREP_TILES], f32)
            nc.sync.dma_start(dinvp_t[:], dinvp_d[:])
            dinvd_t = cpool.tile([128, c.NW], f32)
            nc.sync.dma_start(dinvd_t[:], dinvd_d[:])
            dinvd2_t = cpool.tile([128, c.NW], f32)
            nc.sync.dma_start(dinvd2_t[:], dinvd2_d[:])
            pidx_t = cpool.tile([128, p.PREP_ROWS // 16], i16)
            nc.sync.dma_start(pidx_t[:], prep_idx_d[:])

            qn = 0

            def emit_ag(piece, b_tile, f_tiles, calls_per_piece):
                ins_ap = b_tile[piece * calls_per_piece:(piece + 1) * calls_per_piece]
                nc.gpsimd.collective_compute(
                    "AllGather", mybir.AluOpType.bypass,
                    ins=[ins_ap.flatten_outer_dims()],
                    outs=[f_tiles[piece][:]],
                    replica_groups=[list(range(c.NC))],
                )

            # ---------------- prep: g1 = dinv * (embed[tok] @ A1.T) ----------------
            emb_bc = embed.bitcast(bf16)          # [V, 2*DIN]
            halfA = emb_bc[0:c.HALF, :]
            halfB = emb_bc[c.HALF:c.V, :]
            n_callsA = p.NA // c.PREP_CALL
            ag1_next = 0
            for call in range(p.N_CALLS):
                # interleave AG1 pieces one call after their inputs complete
                if ag1_next < c.NPIECE and call == CALLS_PP * (ag1_next + 1) + 1:
                    emit_ag(ag1_next, g1b, g1f, CALLS_PP)
                    ag1_next += 1
                srcap = halfA if call < n_callsA else halfB
                et = embpool.tile([128, TPC, 2 * DIN], bf16, tag="emb")
                nc.gpsimd.dma_gather(
                    et[:], srcap,
                    pidx_t[:, call * (c.PREP_CALL // 16):(call + 1) * (c.PREP_CALL // 16)],
                    num_idxs=c.PREP_CALL, num_idxs_reg=c.PREP_CALL, elem_size=2 * DIN,
                    queue_num=qn % c.NQ,
                )
                qn += 1
                etf = et[:].bitcast(f32)          # [128, TPC, DIN]
                xb = postpool.tile([128, TPC, DIN], bf16, tag="xb")
                nc.scalar.copy(xb[:], etf[:, :, :])
                pbuf = postpool.tile([128, TPC, D], bf16, tag="pb")
                for j in range(TPC):
                    t_idx = call * TPC + j
                    mp = zpool.tile([128, D], f32, tag="z", name=f"mp_{t_idx}")
                    for kk in range(KIN):
                        tp = auxpool.tile([128, 128], bf16, tag="aux")
                        nc.tensor.transpose(tp[:], xb[:, j, kk * 128:(kk + 1) * 128], ident_t[:])
                        xT = postpool.tile([128, 128], bf16, tag="xT")
                        nc.vector.tensor_copy(xT[:], tp[:])
                        nc.tensor.matmul(mp[:], xT[:], a1t_t[:, kk, :],
                                         start=(kk == 0), stop=(kk == KIN - 1))
                    nc.scalar.activation(pbuf[:, j, :], mp[:], act_copy,
                                         scale=dinvp_t[:, t_idx:t_idx + 1])
                nc.sync.dma_start(g1b[call], pbuf[:])
            while ag1_next < c.NPIECE:
                emit_ag(ag1_next, g1b, g1f, CALLS_PP)
                ag1_next += 1

            # ---------------- layers ----------------
            def layer(sched, st_d, tables, REG, is_last):
                groups, calls, offs = sched["groups"], sched["calls"], sched["offs"]
                blk_w, first_blk, last_blk = sched["blk_w"], sched["first_blk"], sched["last_blk"]
                nonlocal qn
                call_i = 0
                ncalls = len(calls)
                ag2_next = 0
                zt = {}
                for gi, grp in enumerate(groups):
                    if (not is_last) and ag2_next < c.NPIECE and gi == GRP_PP * (ag2_next + 1) + 1:
                        emit_ag(ag2_next, g2b, g2f, GRP_PP)
                        ag2_next += 1
                    for w in grp:
                        zt[w] = zpool.tile([128, D], f32, tag="z", name=f"zt_w{w}")
                    while call_i < ncalls and calls[call_i][0] == gi:
                        _, r, b0, nbc, nreg = calls[call_i]
                        off = offs[call_i]
                        call_i += 1
                        st_t = iopool.tile([128, c.CALLBLK * 10], i16, tag="st")
                        nc.sync.dma_start(st_t[:, 0:nbc * 10], st_d[:, off:off + nbc * 10])
                        colb = st_t[:, nbc * 8:nbc * 10].bitcast(f32)
                        P, h = r // c.RPP, r % c.RPP
                        table = tables[P][h * REG:(h + 1) * REG, :]
                        msg_t = msgpool.tile([128, c.CALLBLK, D], bf16, tag="msg")
                        nc.gpsimd.dma_gather(
                            msg_t[:, 0:nbc, :], table, st_t[:, 0:nbc * 8],
                            num_idxs=nbc * 128, num_idxs_reg=nreg, elem_size=D,
                            queue_num=qn % c.NQ,
                        )
                        qn += 1
                        for b in range(nbc):
                            gb = b0 + b
                            w = int(blk_w[gb])
                            kl = min(128, max(16, nreg - b * 128))
                            s_t = spool.tile([128, D], bf16, tag="s")
                            nc.vector.tensor_scalar(
                                out=s_t[:], in0=iota_t[:], scalar1=colb[:, b:b + 1],
                                scalar2=None, op0=mybir.AluOpType.is_equal,
                            )
                            nc.tensor.matmul(zt[w][:], s_t[0:kl, :], msg_t[0:kl, b, :],
                                             start=(gb == first_blk[w]), stop=(gb == last_blk[w]))
                    # post-process this group's windows
                    if not is_last:
                        gbuf = grppool.tile([128, SGW, D], bf16, tag="gb")
                        for wg, w in enumerate(grp):
                            p1 = postpool.tile([128, D], bf16, tag="p1")
                            nc.scalar.activation(p1[:], zt[w][:], relu,
                                                 scale=dinvd2_t[:, w:w + 1])
                            tp = auxpool.tile([128, 128], bf16, tag="aux")
                            nc.tensor.transpose(tp[:], p1[:], ident_t[:])
                            p1T = postpool.tile([128, D], bf16, tag="p1T")
                            nc.vector.tensor_copy(p1T[:], tp[:])
                            gp = auxpool.tile([128, D], f32, tag="aux")
                            nc.tensor.matmul(gp[:], p1T[:], w2t_t[:], start=True, stop=True)
                            nc.scalar.copy(gbuf[:, wg, :], gp[:])
                            del zt[w]
                        nc.sync.dma_start(g2b[gi], gbuf[:])
                    else:
                        obuf = grppool.tile([128, SGW, D], f32, tag="ob")
                        for wg, w in enumerate(grp):
                            nc.scalar.activation(obuf[:, wg, :], zt[w][:], act_copy,
                                                 scale=dinvd_t[:, w:w + 1])
                            del zt[w]
                        nc.sync.dma_start(outp[gi], obuf[:])
                if not is_last:
                    while ag2_next < c.NPIECE:
                        emit_ag(ag2_next, g2b, g2f, GRP_PP)
                        ag2_next += 1

            layer(p.s1, st1_d, g1f, p.REG1, is_last=False)
            layer(p.s2, st2_d, g2f, p.REG2, is_last=True)
    nc.finalize()
    return nc


# ---------------- host-side weight prep + in_maps ----------------

def make_in_maps(p, embed_table, W_node_w, W_node_b, conv1_w, conv1_b, conv2_w, conv2_b):
    c = p.cfg
    assert np.abs(W_node_b).max() == 0 and np.abs(conv1_b).max() == 0 and np.abs(conv2_b).max() == 0, \
        "nonzero biases not supported by this build (all-zero in this problem)"
    A1 = (np.asarray(conv1_w, np.float64) @ np.asarray(W_node_w, np.float64)).astype(np.float32)
    a1t = np.ascontiguousarray(A1.T).astype(BF)                  # [DIN, D]
    w2t = np.ascontiguousarray(np.asarray(conv2_w, np.float32).T).astype(BF)
    iota = np.tile(np.arange(c.D, dtype=np.float32), (128, 1)).astype(BF)
    ident = np.eye(128, dtype=np.float32).astype(BF)
    emb = np.ascontiguousarray(np.asarray(embed_table, np.float32))
    maps = []
    for k in range(c.NC):
        maps.append({
            "embed": emb,
            "prep_idx": p.prep_idx[k],
            "st1": p.st1[k], "st2": p.st2[k],
            "dinvp": p.dinv_p[k], "dinvd": p.dinv_d[k],
            "dinvd2": p.dinv_d[k] ** 2,
            "a1t": a1t, "w2t": w2t, "iota": iota, "ident": ident,
        })
    return maps


def assemble(p, results):
    c = p.cfg
    out = np.empty((c.N, c.D), np.float32)
    for k in range(c.NC):
        r = results[k]["out"]          # [NGRP, 128, SGW, D]
        r2 = r.transpose(0, 2, 1, 3).reshape(c.NW * 128, c.D)
        out[k * c.NPC + p.order_d[k]] = r2[: c.NPC]
    return out


_CACHE = {}

def kernel(node_tokens, edge_index, embed_table, W_node_w, W_node_b,
           conv1_w, conv1_b, conv2_w, conv2_b):
    cfg = FULL
    p = preprocess(cfg, node_tokens, edge_index)
    key = "full"
    if key not in _CACHE:
        _CACHE[key] = build_nc(p)
    nc = _CACHE[key]
    maps = make_in_maps(p, embed_table, W_node_w, W_node_b, conv1_w, conv1_b, conv2_w, conv2_b)
    res = run_bass_kernel_spmd(nc, maps, core_ids=list(range(cfg.NC)))
    return assemble(p, res.results)
